# revision 3
# baseline (speedup 1.0000x reference)
"""Trainium2 Bass kernel for the 5-layer dilated sparse-conv encoder.

Network (per batch): 1ch -> [3x3x3 dil1] -> 2ch -> [3x3x3 dil2] -> 2ch
-> [3x3x3 dil4] -> 2ch -> [3x3x3 dil2] -> 2ch -> [1x1x1] -> sigmoid,
with relu+occupancy-mask after each hidden conv and mask after sigmoid.

Sharding: 8 cores = 2 batches x 4 z-slabs of 48 planes. Each core gets a
66-plane input slab (z halo 9) and computes its 48 output planes with no
cross-core communication.

Per-core algorithm: contraction over z on the TensorEngine. Every layer's
activation lives in SBUF as [2ch*64 partitions, y_window, 192] where the
64-row z-window is [z0-8, z1+8). A conv layer is 9 PSUM-accumulated
matmuls (one per (dy,dx) tap, shifted free-dim access patterns) against
host-built banded weight matrices that fold the 3 dz taps and both
channels into the contraction. Zero 'SAME' padding at volume borders
falls out of PSUM's has_written bits via per-tap restricted output
rectangles. relu+mask is one fused scalar_tensor_tensor DVE op.
"""

import os
import sys

import numpy as np


def _ensure_import_path():
    for p in ("/opt/trn_rl_repo", "/root/.axon_site/_ro/trn_rl_repo"):
        if os.path.isdir(p) and p not in sys.path:
            sys.path.insert(0, p)


_ensure_import_path()

import concourse.mybir as mybir  # noqa: E402
import concourse.tile as tile  # noqa: E402
from concourse import bacc, bass_utils  # noqa: E402

B, D = 2, 192
ZS = 48  # z planes per core
HZ = 9  # input z halo
ZIN = ZS + 2 * HZ  # 66 input planes per core
ZW = 64  # uniform stored z-window [z0-8, z1+8)
TAPS = [(0, 0)] + [
    (dy, dx) for dy in (-1, 0, 1) for dx in (-1, 0, 1) if (dy, dx) != (0, 0)
]
# (dilation, valid z-window in 64-coords) per layer
LAYERS = [(1, 0, 64), (2, 2, 62), (4, 6, 58), (2, 8, 56)]
V5 = (8, 56)

ACT_DT = "float32"  # activation/storage dtype for conv tiles
YBLK = 24  # output-y rows per block


def _clip(a, b):
    return max(a, 0), min(b, D)


def _build_bands(W1, W2, W3, W4, W5):
    """Banded lhsT matrices, one per (layer, tap). Returns dict of arrays."""
    Ws = [np.asarray(w, np.float32) for w in (W1, W2, W3, W4)]
    out = {}
    # L1: [66, 9, 128]  (Cin=1)
    b1 = np.zeros((ZIN, 9, 128), np.float32)
    for t, (dy, dx) in enumerate(TAPS):
        for co in range(2):
            for dz in (-1, 0, 1):
                for zr in range(64):
                    b1[zr + 1 + dz, t, co * 64 + zr] = Ws[0][
                        co, 0, dz + 1, dy + 1, dx + 1
                    ]
    out["b1"] = b1
    # L2..L4: [128, 9, 128]
    for li, (d, a, b) in enumerate(LAYERS[1:], start=2):
        w = Ws[li - 1]
        bb = np.zeros((128, 9, 128), np.float32)
        for t, (dy, dx) in enumerate(TAPS):
            for co in range(2):
                for ci in range(2):
                    for dz in (-1, 0, 1):
                        for zr in range(a, b):
                            bb[ci * 64 + zr + d * dz, t, co * 64 + zr] = w[
                                co, ci, dz + 1, dy + 1, dx + 1
                            ]
        out[f"b{li}"] = bb
    # L5: [128, 128]
    w5 = np.asarray(W5, np.float32)
    b5 = np.zeros((128, 128), np.float32)
    for co in range(2):
        for ci in range(2):
            for zr in range(V5[0], V5[1]):
                b5[ci * 64 + zr, co * 64 + zr] = w5[co, ci, 0, 0, 0]
    out["b5"] = b5
    return out


def _conv_layer(nc, ps, src, src_y0, K, bt, d, a, b, wy, mk, mk_y0, dst, relu_mask=True):
    """One conv layer for one y-block: 9-tap banded matmuls + fused epilogue.

    src: source tile AP base [K partitions, src_ylen, 192], covering global
    y rows starting at src_y0. dst covers wy=(w0,w1). mk covers mask window
    starting at mk_y0.
    """
    w0, w1 = wy
    taps = TAPS if bt.shape[1] == 9 else [(0, 0)]
    for ys in range(w0, w1, 8):
        ye = min(ys + 8, w1)
        for xs in range(0, D, 64):
            xe = xs + 64
            acc = ps.tile([128, 8, 64], mybir.dt.float32, tag="psum")
            live = []
            for t, (dy, dx) in enumerate(taps):
                oy0, oy1 = max(ys, -dy * d), min(ye, D - dy * d)
                ox0, ox1 = max(xs, -dx * d), min(xe, D - dx * d)
                if oy0 < oy1 and ox0 < ox1:
                    live.append((t, dy, dx, oy0, oy1, ox0, ox1))
            assert live[0][0] == 0  # center tap first, covers full rect
            for i, (t, dy, dx, oy0, oy1, ox0, ox1) in enumerate(live):
                sy0, sy1 = oy0 + dy * d - src_y0, oy1 + dy * d - src_y0
                assert 0 <= sy0 < sy1 <= src.shape[1], (sy0, sy1, src.shape)
                nc.tensor.matmul(
                    acc[:, oy0 - ys : oy1 - ys, ox0 - xs : ox1 - xs],
                    bt[0:K, t, :] if bt.shape[1] == 9 else bt[0:K, :],
                    src[0:K, sy0:sy1, ox0 + dx * d : ox1 + dx * d],
                    start=(i == 0),
                    stop=(i == len(live) - 1),
                )
            cy = ye - ys
            if relu_mask:
                nc.vector.scalar_tensor_tensor(
                    dst[:, ys - w0 : ye - w0, xs:xe],
                    acc[:, 0:cy, :],
                    0.0,
                    mk[:, ys - mk_y0 : ye - mk_y0, xs:xe],
                    op0=mybir.AluOpType.max,
                    op1=mybir.AluOpType.mult,
                )
            else:  # L5: sigmoid then mask
                nc.scalar.activation(
                    dst[:, ys - w0 : ye - w0, xs:xe],
                    acc[:, 0:cy, :],
                    mybir.ActivationFunctionType.Sigmoid,
                )
                nc.vector.tensor_tensor(
                    dst[:, ys - w0 : ye - w0, xs:xe],
                    dst[:, ys - w0 : ye - w0, xs:xe],
                    mk[:, ys - mk_y0 : ye - mk_y0, xs:xe],
                    op=mybir.AluOpType.mult,
                )


def build_program():
    dt = getattr(mybir.dt, ACT_DT)
    f32 = mybir.dt.float32
    nc = bacc.Bacc("TRN2", target_bir_lowering=False, debug=False)

    xslab = nc.dram_tensor("xslab", [ZIN, D, D], f32, kind="ExternalInput")
    b1d = nc.dram_tensor("b1", [ZIN, 9, 128], f32, kind="ExternalInput")
    b2d = nc.dram_tensor("b2", [128, 9, 128], f32, kind="ExternalInput")
    b3d = nc.dram_tensor("b3", [128, 9, 128], f32, kind="ExternalInput")
    b4d = nc.dram_tensor("b4", [128, 9, 128], f32, kind="ExternalInput")
    b5d = nc.dram_tensor("b5", [128, 128], f32, kind="ExternalInput")
    prob_o = nc.dram_tensor("prob_o", [ZS, D, D], f32, kind="ExternalOutput")
    regr_o = nc.dram_tensor("regr_o", [ZS, D, D], f32, kind="ExternalOutput")

    with tile.TileContext(nc) as tc:
        with (
            tc.tile_pool(name="wpool", bufs=1) as wp,
            tc.tile_pool(name="act", bufs=1) as ap,
            tc.tile_pool(name="ps", bufs=8, space="PSUM") as ps,
        ):
            b1t = wp.tile([ZIN, 9, 128], f32)
            b2t = wp.tile([128, 9, 128], f32)
            b3t = wp.tile([128, 9, 128], f32)
            b4t = wp.tile([128, 9, 128], f32)
            b5t = wp.tile([128, 128], f32)
            for t, dram in ((b1t, b1d), (b2t, b2d), (b3t, b3d), (b4t, b4d), (b5t, b5d)):
                nc.sync.dma_start(t[:], dram[:])

            for y0 in range(0, D, YBLK):
                y1 = y0 + YBLK
                win = _clip(y0 - HZ, y1 + HZ)
                w1y = _clip(y0 - 8, y1 + 8)
                w2y = _clip(y0 - 6, y1 + 6)
                w3y = _clip(y0 - 2, y1 + 2)
                w4y = (y0, y1)

                xt = ap.tile([ZIN, YBLK + 2 * HZ, D], f32, tag="xt")
                nc.sync.dma_start(
                    xt[:, 0 : win[1] - win[0], :], xslab[:, win[0] : win[1], :]
                )
                # mask: duplicate raw data planes [z0-8, z1+8) into both
                # channel halves, then (x != 0) in place
                mlen = w1y[1] - w1y[0]
                mo = w1y[0] - win[0]
                mk = ap.tile([128, YBLK + 16, D], f32, tag="mk")
                nc.sync.dma_start(mk[0:64, 0:mlen, :], xt[1:65, mo : mo + mlen, :])
                nc.sync.dma_start(mk[64:128, 0:mlen, :], xt[1:65, mo : mo + mlen, :])
                nc.vector.tensor_scalar(
                    mk[:, 0:mlen, :], mk[:, 0:mlen, :], 0.0, None,
                    op0=mybir.AluOpType.not_equal,
                )

                l1 = ap.tile([128, YBLK + 16, D], dt, tag="l1")
                _conv_layer(nc, ps, xt[:], win[0], ZIN, b1t[:], 1, 0, 64,
                            w1y, mk[:], w1y[0], l1[:])
                l2 = ap.tile([128, YBLK + 12, D], dt, tag="l2")
                _conv_layer(nc, ps, l1[:], w1y[0], 128, b2t[:], 2, 2, 62,
                            w2y, mk[:], w1y[0], l2[:])
                l3 = ap.tile([128, YBLK + 4, D], dt, tag="l3")
                _conv_layer(nc, ps, l2[:], w2y[0], 128, b3t[:], 4, 6, 58,
                            w3y, mk[:], w1y[0], l3[:])
                l4 = ap.tile([128, YBLK, D], dt, tag="l4")
                _conv_layer(nc, ps, l3[:], w3y[0], 128, b4t[:], 2, 8, 56,
                            w4y, mk[:], w1y[0], l4[:])
                ot = ap.tile([128, YBLK, D], f32, tag="l3")  # share l3 slot
                _conv_layer(nc, ps, l4[:], w4y[0], 128, b5t[:], 1, 8, 56,
                            w4y, mk[:], w1y[0], ot[:], relu_mask=False)

                nc.sync.dma_start(prob_o[:, y0:y1, :], ot[8:56, :, :])
                nc.sync.dma_start(regr_o[:, y0:y1, :], ot[72:120, :, :])

    nc.compile()
    return nc


_prog_cache = {}


def kernel(data, W1, W2, W3, W4, W5):
    _ensure_import_path()
    data = np.asarray(data, np.float32)
    if "nc" not in _prog_cache:
        _prog_cache["nc"] = build_program()
    nc = _prog_cache["nc"]

    bands = _build_bands(W1, W2, W3, W4, W5)
    dpad = np.zeros((B, D + 2 * HZ, D, D), np.float32)
    dpad[:, HZ : HZ + D] = data
    in_maps = []
    for c in range(8):
        bi, s = c // 4, c % 4
        in_maps.append(
            dict(xslab=np.ascontiguousarray(dpad[bi, s * ZS : s * ZS + ZIN]), **bands)
        )

    res = bass_utils.run_bass_kernel_spmd(nc, in_maps, list(range(8))).results

    prob = np.zeros((B, 1, D, D, D), np.float32)
    regr = np.zeros((B, 1, D, D, D), np.float32)
    for c in range(8):
        bi, s = c // 4, c % 4
        prob[bi, 0, s * ZS : (s + 1) * ZS] = res[c]["prob_o"]
        regr[bi, 0, s * ZS : (s + 1) * ZS] = res[c]["regr_o"]
    return (prob, regr)


# revision 14
# speedup vs baseline: 2.6438x; 2.6438x over previous
"""Trainium2 Bass kernel for the 5-layer dilated sparse-conv encoder.

Network (per batch): 1ch -> [3x3x3 dil1] -> 2ch -> [3x3x3 dil2] -> 2ch
-> [3x3x3 dil4] -> 2ch -> [3x3x3 dil2] -> 2ch -> [1x1x1] -> sigmoid,
with relu+occupancy-mask after each hidden conv and mask after sigmoid.

Sharding: 8 cores = 2 batches x 4 z-slabs of 48 planes. Each core gets a
66-plane input slab (z halo 9) and computes its 48 output planes with no
cross-core communication.

Per-core algorithm: contraction over z on the TensorEngine. Every layer's
activation lives in SBUF as [2ch*64 partitions, y_window, 192] where the
64-row z-window is [z0-8, z1+8). A conv layer is 9 PSUM-accumulated
matmuls (one per (dy,dx) tap, shifted free-dim access patterns) against
host-built banded weight matrices that fold the 3 dz taps and both
channels into the contraction. Zero 'SAME' padding at volume borders
falls out of PSUM's has_written bits via per-tap restricted output
rectangles. relu+mask is one fused scalar_tensor_tensor DVE op.
"""

import os
import sys

import numpy as np


def _ensure_import_path():
    for p in ("/opt/trn_rl_repo", "/root/.axon_site/_ro/trn_rl_repo"):
        if os.path.isdir(p) and p not in sys.path:
            sys.path.insert(0, p)


_ensure_import_path()

import concourse.mybir as mybir  # noqa: E402
import concourse.tile as tile  # noqa: E402
from concourse import bacc, bass_utils  # noqa: E402

B, D = 2, 192
ZS = 48  # z planes per core
HZ = 9  # input z halo
ZIN = ZS + 2 * HZ  # 66 input planes per core
ZW = 64  # uniform stored z-window [z0-8, z1+8)
TAPS = [(0, 0)] + [
    (dy, dx) for dy in (-1, 0, 1) for dx in (-1, 0, 1) if (dy, dx) != (0, 0)
]
# (dilation, valid z-window in 64-coords) per layer
LAYERS = [(1, 0, 64), (2, 2, 62), (4, 6, 58), (2, 8, 56)]
V5 = (8, 56)

ACT_DT = "float32r"  # activation/storage dtype for conv tiles (full-rate PE)
YBLK = 24  # output-y rows per block


def _clip(a, b):
    return max(a, 0), min(b, D)


def _build_bands(W1, W2, W3, W4, W5):
    """Banded lhsT matrices, one per (layer, tap). Returns dict of arrays."""
    Ws = [np.asarray(w, np.float32) for w in (W1, W2, W3, W4)]
    out = {}
    # L1: [66, 9, 128]  (Cin=1)
    b1 = np.zeros((ZIN, 9, 128), np.float32)
    for t, (dy, dx) in enumerate(TAPS):
        for co in range(2):
            for dz in (-1, 0, 1):
                for zr in range(64):
                    b1[zr + 1 + dz, t, co * 64 + zr] = Ws[0][
                        co, 0, dz + 1, dy + 1, dx + 1
                    ]
    out["b1"] = b1
    # L2..L4: [128, 9, 128]
    for li, (d, a, b) in enumerate(LAYERS[1:], start=2):
        w = Ws[li - 1]
        bb = np.zeros((128, 9, 128), np.float32)
        for t, (dy, dx) in enumerate(TAPS):
            for co in range(2):
                for ci in range(2):
                    for dz in (-1, 0, 1):
                        for zr in range(a, b):
                            bb[ci * 64 + zr + d * dz, t, co * 64 + zr] = w[
                                co, ci, dz + 1, dy + 1, dx + 1
                            ]
        out[f"b{li}"] = bb
    # L5: [128, 128]
    w5 = np.asarray(W5, np.float32)
    b5 = np.zeros((128, 128), np.float32)
    for co in range(2):
        for ci in range(2):
            for zr in range(V5[0], V5[1]):
                b5[ci * 64 + zr, co * 64 + zr] = w5[co, ci, 0, 0, 0]
    out["b5"] = b5
    return out


def _conv_layer(nc, ps, src, src_y0, K, bt, d, a, b, wy, mk, mk_y0, dst,
                relu_mask=True, xpad=0):
    """One conv layer for one y-block: 9-tap banded matmuls + fused epilogue.

    src: source tile AP base [K partitions, src_ylen, 192 + 2*xpad], covering
    global y rows starting at src_y0 (x col j holds global x = j - xpad; with
    xpad the border columns are zeros and x is never range-restricted, which
    keeps fp32r ISA alignment happy for odd dilation shifts). dst covers
    wy=(w0,w1). mk covers mask window starting at mk_y0.
    """
    w0, w1 = wy
    taps = TAPS if bt.shape[1] == 9 else [(0, 0)]
    for ys in range(w0, w1, 8):
        ye = min(ys + 8, w1)
        for xs in range(0, D, 64):
            xe = xs + 64
            acc = ps.tile([128, 8, 64], mybir.dt.float32, tag="psum")
            live = []
            for t, (dy, dx) in enumerate(taps):
                oy0, oy1 = max(ys, -dy * d), min(ye, D - dy * d)
                if xpad:
                    ox0, ox1 = xs, xe
                else:
                    ox0, ox1 = max(xs, -dx * d), min(xe, D - dx * d)
                if oy0 < oy1 and ox0 < ox1:
                    live.append((t, dy, dx, oy0, oy1, ox0, ox1))
            assert live[0][0] == 0  # center tap first, covers full rect
            for i, (t, dy, dx, oy0, oy1, ox0, ox1) in enumerate(live):
                sy0, sy1 = oy0 + dy * d - src_y0, oy1 + dy * d - src_y0
                assert 0 <= sy0 < sy1 <= src.shape[1], (sy0, sy1, src.shape)
                sx0 = ox0 + dx * d + xpad
                assert 0 <= sx0 and sx0 + (ox1 - ox0) <= src.shape[2]
                nc.tensor.matmul(
                    acc[:, oy0 - ys : oy1 - ys, ox0 - xs : ox1 - xs],
                    bt[0:K, t, :] if bt.shape[1] == 9 else bt[0:K, :],
                    src[0:K, sy0:sy1, sx0 : sx0 + (ox1 - ox0)],
                    start=(i == 0),
                    stop=(i == len(live) - 1),
                )
            cy = ye - ys
            if relu_mask:
                nc.vector.scalar_tensor_tensor(
                    dst[:, ys - w0 : ye - w0, xs:xe],
                    acc[:, 0:cy, :],
                    0.0,
                    mk[:, ys - mk_y0 : ye - mk_y0, xs:xe],
                    op0=mybir.AluOpType.max,
                    op1=mybir.AluOpType.mult,
                )
            else:  # L5: sigmoid then mask
                nc.scalar.activation(
                    dst[:, ys - w0 : ye - w0, xs:xe],
                    acc[:, 0:cy, :],
                    mybir.ActivationFunctionType.Sigmoid,
                )
                nc.vector.tensor_tensor(
                    dst[:, ys - w0 : ye - w0, xs:xe],
                    dst[:, ys - w0 : ye - w0, xs:xe],
                    mk[:, ys - mk_y0 : ye - mk_y0, xs:xe],
                    op=mybir.AluOpType.mult,
                )


def build_program():
    dt = getattr(mybir.dt, ACT_DT)
    f32 = mybir.dt.float32
    nc = bacc.Bacc("TRN2", target_bir_lowering=False, debug=False)

    xslab = nc.dram_tensor("xslab", [ZIN, D, D + 2], dt, kind="ExternalInput")
    b1d = nc.dram_tensor("b1", [ZIN, 9, 128], dt, kind="ExternalInput")
    b2d = nc.dram_tensor("b2", [128, 9, 128], dt, kind="ExternalInput")
    b3d = nc.dram_tensor("b3", [128, 9, 128], dt, kind="ExternalInput")
    b4d = nc.dram_tensor("b4", [128, 9, 128], dt, kind="ExternalInput")
    b5d = nc.dram_tensor("b5", [128, 128], dt, kind="ExternalInput")
    prob_o = nc.dram_tensor("prob_o", [ZS, D, D], f32, kind="ExternalOutput")
    regr_o = nc.dram_tensor("regr_o", [ZS, D, D], f32, kind="ExternalOutput")

    with tile.TileContext(nc) as tc:
        with (
            tc.tile_pool(name="wpool", bufs=1) as wp,
            tc.tile_pool(name="act", bufs=1) as ap,
            tc.tile_pool(name="ps", bufs=8, space="PSUM") as ps,
        ):
            b1t = wp.tile([ZIN, 9, 128], dt)
            b2t = wp.tile([128, 9, 128], dt)
            b3t = wp.tile([128, 9, 128], dt)
            b4t = wp.tile([128, 9, 128], dt)
            b5t = wp.tile([128, 128], dt)
            for t, dram in ((b1t, b1d), (b2t, b2d), (b3t, b3d), (b4t, b4d), (b5t, b5d)):
                nc.sync.dma_start(t[:], dram[:])

            for y0 in range(0, D, YBLK):
                y1 = y0 + YBLK
                win = _clip(y0 - HZ, y1 + HZ)
                w1y = _clip(y0 - 8, y1 + 8)
                w2y = _clip(y0 - 6, y1 + 6)
                w3y = _clip(y0 - 2, y1 + 2)
                w4y = (y0, y1)

                xt = ap.tile([ZIN, YBLK + 2 * HZ, D + 2], dt, tag="xt")
                wlen = win[1] - win[0]
                nc.sync.dma_start(
                    xt[:, 0:wlen, :], xslab[:, win[0] : win[1], :]
                )
                # mask: duplicate raw data planes [z0-8, z1+8) into both
                # channel halves, then (x != 0) in place
                mlen = w1y[1] - w1y[0]
                mo = w1y[0] - win[0]
                mk = ap.tile([128, YBLK + 16, D], f32, tag="mk")
                xsrc = xt[1:65, mo : mo + mlen, 1 : D + 1].bitcast(f32)
                nc.sync.dma_start(mk[0:64, 0:mlen, :], xsrc)
                nc.sync.dma_start(mk[64:128, 0:mlen, :], xsrc)
                nc.vector.tensor_scalar(
                    mk[:, 0:mlen, :], mk[:, 0:mlen, :], 0.0, None,
                    op0=mybir.AluOpType.not_equal,
                )

                l1 = ap.tile([128, YBLK + 16, D], dt, tag="l1")
                _conv_layer(nc, ps, xt[:], win[0], ZIN, b1t[:], 1, 0, 64,
                            w1y, mk[:], w1y[0], l1[:], xpad=1)
                l2 = ap.tile([128, YBLK + 12, D], dt, tag="l2")
                _conv_layer(nc, ps, l1[:], w1y[0], 128, b2t[:], 2, 2, 62,
                            w2y, mk[:], w1y[0], l2[:])
                l3 = ap.tile([128, YBLK + 4, D], dt, tag="l3")
                _conv_layer(nc, ps, l2[:], w2y[0], 128, b3t[:], 4, 6, 58,
                            w3y, mk[:], w1y[0], l3[:])
                l4 = ap.tile([128, YBLK, D], dt, tag="l4")
                _conv_layer(nc, ps, l3[:], w3y[0], 128, b4t[:], 2, 8, 56,
                            w4y, mk[:], w1y[0], l4[:])
                ot = ap.tile([128, YBLK, D], f32, tag="l3")  # share l3 slot
                _conv_layer(nc, ps, l4[:], w4y[0], 128, b5t[:], 1, 8, 56,
                            w4y, mk[:], w1y[0], ot[:], relu_mask=False)

                nc.sync.dma_start(prob_o[:, y0:y1, :], ot[8:56, :, :])
                nc.sync.dma_start(regr_o[:, y0:y1, :], ot[72:120, :, :])

    nc.compile()
    return nc


_prog_cache = {}


def make_in_maps(data, W1, W2, W3, W4, W5):
    bands = _build_bands(W1, W2, W3, W4, W5)
    dpad = np.zeros((B, D + 2 * HZ, D, D + 2), np.float32)
    dpad[:, HZ : HZ + D, :, 1 : D + 1] = data
    in_maps = []
    for c in range(8):
        bi, s = c // 4, c % 4
        in_maps.append(
            dict(xslab=np.ascontiguousarray(dpad[bi, s * ZS : s * ZS + ZIN]), **bands)
        )
    return in_maps


def kernel(data, W1, W2, W3, W4, W5):
    _ensure_import_path()
    data = np.asarray(data, np.float32)
    if "nc" not in _prog_cache:
        _prog_cache["nc"] = build_program()
    nc = _prog_cache["nc"]

    in_maps = make_in_maps(data, W1, W2, W3, W4, W5)
    res = bass_utils.run_bass_kernel_spmd(nc, in_maps, list(range(8))).results

    prob = np.zeros((B, 1, D, D, D), np.float32)
    regr = np.zeros((B, 1, D, D, D), np.float32)
    for c in range(8):
        bi, s = c // 4, c % 4
        prob[bi, 0, s * ZS : (s + 1) * ZS] = res[c]["prob_o"]
        regr[bi, 0, s * ZS : (s + 1) * ZS] = res[c]["regr_o"]
    return (prob, regr)


# revision 19
# speedup vs baseline: 4.3774x; 1.6557x over previous
"""Trainium2 Bass kernel for the 5-layer dilated sparse-conv encoder.

Network (per batch): 1ch -> [3x3x3 dil1] -> 2ch -> [3x3x3 dil2] -> 2ch
-> [3x3x3 dil4] -> 2ch -> [3x3x3 dil2] -> 2ch -> [1x1x1] -> sigmoid,
with relu+occupancy-mask after each hidden conv and mask after sigmoid.

Sharding: 8 cores = 2 batches x 4 z-slabs of 48 planes. Each core gets a
66-plane input slab (z halo 9) and computes its 48 output planes with no
cross-core communication.

Per-core algorithm: contraction over z on the TensorEngine. Every layer's
activation lives in SBUF as [2ch*64 partitions, y_window, 192] where the
64-row z-window is [z0-8, z1+8). A conv layer is 9 PSUM-accumulated
matmuls (one per (dy,dx) tap, shifted free-dim access patterns) against
host-built banded weight matrices that fold the 3 dz taps and both
channels into the contraction. Zero 'SAME' padding at volume borders
falls out of PSUM's has_written bits via per-tap restricted output
rectangles. relu+mask is one fused scalar_tensor_tensor DVE op.
"""

import os
import sys

import numpy as np


def _ensure_import_path():
    for p in ("/opt/trn_rl_repo", "/root/.axon_site/_ro/trn_rl_repo"):
        if os.path.isdir(p) and p not in sys.path:
            sys.path.insert(0, p)


_ensure_import_path()

import concourse.mybir as mybir  # noqa: E402
import concourse.tile as tile  # noqa: E402
from concourse import bacc, bass_utils  # noqa: E402

B, D = 2, 192
ZS = 48  # z planes per core
HZ = 9  # input z halo
ZIN = ZS + 2 * HZ  # 66 input planes per core
ZW = 64  # uniform stored z-window [z0-8, z1+8)
TAPS = [(0, 0)] + [
    (dy, dx) for dy in (-1, 0, 1) for dx in (-1, 0, 1) if (dy, dx) != (0, 0)
]
# (dilation, valid z-window in 64-coords) per layer
LAYERS = [(1, 0, 64), (2, 2, 62), (4, 6, 58), (2, 8, 56)]
V5 = (8, 56)

ACT_DT = "float32r"  # activation/storage dtype for conv tiles (full-rate PE)
YBLK = 16  # output-y rows per wavefront block


def _clip(a, b):
    return max(a, 0), min(b, D)


def _build_bands(W1, W2, W3, W4, W5):
    """Banded lhsT matrices, one per (layer, tap). Returns dict of arrays."""
    Ws = [np.asarray(w, np.float32) for w in (W1, W2, W3, W4)]
    out = {}
    # L1: [66, 9, 128]  (Cin=1)
    b1 = np.zeros((ZIN, 9, 128), np.float32)
    for t, (dy, dx) in enumerate(TAPS):
        for co in range(2):
            for dz in (-1, 0, 1):
                for zr in range(64):
                    b1[zr + 1 + dz, t, co * 64 + zr] = Ws[0][
                        co, 0, dz + 1, dy + 1, dx + 1
                    ]
    out["b1"] = b1
    # L2..L4: [128, 9, 128]
    for li, (d, a, b) in enumerate(LAYERS[1:], start=2):
        w = Ws[li - 1]
        bb = np.zeros((128, 9, 128), np.float32)
        for t, (dy, dx) in enumerate(TAPS):
            for co in range(2):
                for ci in range(2):
                    for dz in (-1, 0, 1):
                        for zr in range(a, b):
                            bb[ci * 64 + zr + d * dz, t, co * 64 + zr] = w[
                                co, ci, dz + 1, dy + 1, dx + 1
                            ]
        out[f"b{li}"] = bb
    # L5: [128, 128]
    w5 = np.asarray(W5, np.float32)
    b5 = np.zeros((128, 128), np.float32)
    for co in range(2):
        for ci in range(2):
            for zr in range(V5[0], V5[1]):
                b5[ci * 64 + zr, co * 64 + zr] = w5[co, ci, 0, 0, 0]
    out["b5"] = b5
    return out


def _conv_layer(nc, ps, src, src_y0, K, bt, d, wy, mk, mk_y0, dst, dst_y0,
                relu_mask=True, xpad=0):
    """One conv layer for one y-block: 9-tap banded matmuls + fused epilogue.

    src: source tile AP [K partitions, ylen, 192 + 2*xpad] whose row 0 is
    global y = src_y0 (x col j holds global x = j - xpad; with xpad the
    border columns are zeros and x is never range-restricted, which keeps
    fp32r ISA alignment happy for odd dilation shifts). dst computes global
    rows wy=(w0,w1), dst tile row 0 = global dst_y0. mk row 0 = mk_y0.
    """
    w0, w1 = wy
    taps = TAPS if bt.shape[1] == 9 else [(0, 0)]
    for ys in range(w0, w1, 8):
        ye = min(ys + 8, w1)
        for xs in range(0, D, 64):
            xe = xs + 64
            acc = ps.tile([128, 8, 64], mybir.dt.float32, tag="psum")
            live = []
            for t, (dy, dx) in enumerate(taps):
                oy0, oy1 = max(ys, -dy * d), min(ye, D - dy * d)
                if xpad:
                    ox0, ox1 = xs, xe
                else:
                    ox0, ox1 = max(xs, -dx * d), min(xe, D - dx * d)
                if oy0 < oy1 and ox0 < ox1:
                    live.append((t, dy, dx, oy0, oy1, ox0, ox1))
            assert live[0][0] == 0  # center tap first, covers full rect
            for i, (t, dy, dx, oy0, oy1, ox0, ox1) in enumerate(live):
                sy0, sy1 = oy0 + dy * d - src_y0, oy1 + dy * d - src_y0
                assert 0 <= sy0 < sy1 <= src.shape[1], (sy0, sy1, src.shape)
                sx0 = ox0 + dx * d + xpad
                assert 0 <= sx0 and sx0 + (ox1 - ox0) <= src.shape[2]
                nc.tensor.matmul(
                    acc[:, oy0 - ys : oy1 - ys, ox0 - xs : ox1 - xs],
                    bt[0:K, t, :] if bt.shape[1] == 9 else bt[0:K, :],
                    src[0:K, sy0:sy1, sx0 : sx0 + (ox1 - ox0)],
                    start=(i == 0),
                    stop=(i == len(live) - 1),
                )
            cy = ye - ys
            if relu_mask:
                nc.vector.scalar_tensor_tensor(
                    dst[:, ys - dst_y0 : ye - dst_y0, xs:xe],
                    acc[:, 0:cy, :],
                    0.0,
                    mk[:, ys - mk_y0 : ye - mk_y0, xs:xe],
                    op0=mybir.AluOpType.max,
                    op1=mybir.AluOpType.mult,
                )
            else:  # L5: sigmoid then mask
                nc.scalar.activation(
                    dst[:, ys - dst_y0 : ye - dst_y0, xs:xe],
                    acc[:, 0:cy, :],
                    mybir.ActivationFunctionType.Sigmoid,
                )
                nc.vector.tensor_tensor(
                    dst[:, ys - dst_y0 : ye - dst_y0, xs:xe],
                    dst[:, ys - dst_y0 : ye - dst_y0, xs:xe],
                    mk[:, ys - mk_y0 : ye - mk_y0, xs:xe],
                    op=mybir.AluOpType.mult,
                )


def build_program():
    dt = getattr(mybir.dt, ACT_DT)
    f32 = mybir.dt.float32
    nc = bacc.Bacc("TRN2", target_bir_lowering=False, debug=False)

    xslab = nc.dram_tensor("xslab", [ZIN, D, D + 2], dt, kind="ExternalInput")
    b1d = nc.dram_tensor("b1", [ZIN, 9, 128], dt, kind="ExternalInput")
    b2d = nc.dram_tensor("b2", [128, 9, 128], dt, kind="ExternalInput")
    b3d = nc.dram_tensor("b3", [128, 9, 128], dt, kind="ExternalInput")
    b4d = nc.dram_tensor("b4", [128, 9, 128], dt, kind="ExternalInput")
    b5d = nc.dram_tensor("b5", [128, 128], dt, kind="ExternalInput")
    prob_o = nc.dram_tensor("prob_o", [ZS, D, D], f32, kind="ExternalOutput")
    regr_o = nc.dram_tensor("regr_o", [ZS, D, D], f32, kind="ExternalOutput")

    # Skewed y-wavefront: layer l's computed frontier leads the output by
    # h_l rows. Block b computes rows [F(h,b-1), F(h,b)) of each layer
    # (YB rows interior, more in the prologue block, fewer in the last).
    # Each layer tile carries a 2*dil_consumer-row tail between blocks.
    HLEAD = {"xt": HZ, "t1": 8, "t2": 6, "t3": 2, "t4": 0}
    DCONS = {"xt": 1, "t1": 2, "t2": 4, "t3": 2, "t4": 0}

    def F(h, b):
        return 0 if b < 0 else min(YBLK * (b + 1) + h, D)

    def origin(h, dc, b):
        return 0 if b == 0 else F(h, b - 1) - 2 * dc

    NB = D // YBLK

    with tile.TileContext(nc) as tc:
        with (
            tc.tile_pool(name="wpool", bufs=1) as wp,
            tc.tile_pool(name="act", bufs=1) as ap,
            tc.tile_pool(name="mkp", bufs=2) as mkp,
            tc.tile_pool(name="otp", bufs=2) as otp,
            tc.tile_pool(name="ps", bufs=8, space="PSUM") as ps,
        ):
            b1t = wp.tile([ZIN, 9, 128], dt)
            b2t = wp.tile([128, 9, 128], dt)
            b3t = wp.tile([128, 9, 128], dt)
            b4t = wp.tile([128, 9, 128], dt)
            b5t = wp.tile([128, 128], dt)
            for t, dram in ((b1t, b1d), (b2t, b2d), (b3t, b3d), (b4t, b4d), (b5t, b5d)):
                nc.sync.dma_start(t[:], dram[:])

            def ext(name):
                return max(YBLK + HLEAD[name], YBLK + 2 * DCONS[name])

            xt = ap.tile([ZIN, ext("xt"), D + 2], dt, tag="xt")
            t1 = ap.tile([128, ext("t1"), D], dt, tag="t1")
            t2 = ap.tile([128, ext("t2"), D], dt, tag="t2")
            t3 = ap.tile([128, ext("t3"), D], dt, tag="t3")
            t4 = ap.tile([128, ext("t4"), D], dt, tag="t4")

            for b in range(NB):
                # tail copies: last 2*dc computed rows -> tile rows [0, 2dc)
                if b > 0:
                    for tl, name, xw in ((xt, "xt", D + 2), (t1, "t1", D),
                                         (t2, "t2", D), (t3, "t3", D)):
                        h, dc = HLEAD[name], DCONS[name]
                        s0 = F(h, b - 1) - 2 * dc - origin(h, dc, b - 1)
                        nc.vector.tensor_copy(
                            tl[:, 0 : 2 * dc, 0:xw], tl[:, s0 : s0 + 2 * dc, 0:xw]
                        )

                # new input rows
                i0, i1 = F(HZ, b - 1), F(HZ, b)
                r0 = i0 - origin(HZ, 1, b)
                nc.sync.dma_start(xt[:, r0 : r0 + i1 - i0, :], xslab[:, i0:i1, :])

                # mask for this block's union of layer windows
                m0, m1 = YBLK * b, min(YBLK * b + YBLK + 8, D)
                mk = mkp.tile([128, YBLK + 8, D], f32, tag="mk")
                nc.sync.dma_start(
                    mk[0:64, 0 : m1 - m0, :],
                    xslab[1:65, m0:m1, 1 : D + 1].bitcast(f32),
                )
                nc.sync.dma_start(
                    mk[64:128, 0 : m1 - m0, :],
                    xslab[1:65, m0:m1, 1 : D + 1].bitcast(f32),
                )
                nc.vector.tensor_scalar(
                    mk[:, 0 : m1 - m0, :], mk[:, 0 : m1 - m0, :], 0.0, None,
                    op0=mybir.AluOpType.not_equal,
                )

                chain = (
                    ("xt", xt, ZIN, b1t, 1, "t1", t1, 1),
                    ("t1", t1, 128, b2t, 2, "t2", t2, 0),
                    ("t2", t2, 128, b3t, 4, "t3", t3, 0),
                    ("t3", t3, 128, b4t, 2, "t4", t4, 0),
                )
                for sname, stile, K, bt, dil, dname, dtile, xp in chain:
                    wy = (F(HLEAD[dname], b - 1), F(HLEAD[dname], b))
                    _conv_layer(
                        nc, ps, stile[:],
                        origin(HLEAD[sname], DCONS[sname], b), K, bt[:], dil,
                        wy, mk[:], m0, dtile[:],
                        origin(HLEAD[dname], DCONS[dname], b), xpad=xp,
                    )

                # L5: 1x1 conv + sigmoid + mask + store, in 8-row groups
                w5 = (F(0, b - 1), F(0, b))
                o4 = origin(0, 0, b)
                for ys in range(w5[0], w5[1], 8):
                    ye = min(ys + 8, w5[1])
                    ot = otp.tile([128, 8, D], f32, tag="ot")
                    for ps0 in range(ys, ye, 2):
                        ps1 = min(ps0 + 2, ye)
                        acc = ps.tile([128, 2, D], mybir.dt.float32, tag="psum")
                        nc.tensor.matmul(
                            acc[:, 0 : ps1 - ps0, :],
                            b5t[:, :],
                            t4[:, ps0 - o4 : ps1 - o4, :],
                            start=True,
                            stop=True,
                        )
                        nc.scalar.activation(
                            ot[:, ps0 - ys : ps1 - ys, :],
                            acc[:, 0 : ps1 - ps0, :],
                            mybir.ActivationFunctionType.Sigmoid,
                        )
                    nc.vector.tensor_tensor(
                        ot[:, 0 : ye - ys, :],
                        ot[:, 0 : ye - ys, :],
                        mk[:, ys - m0 : ye - m0, :],
                        op=mybir.AluOpType.mult,
                    )
                    nc.sync.dma_start(
                        prob_o[:, ys:ye, :], ot[8:56, 0 : ye - ys, :]
                    )
                    nc.sync.dma_start(
                        regr_o[:, ys:ye, :], ot[72:120, 0 : ye - ys, :]
                    )

    nc.compile()
    return nc


_prog_cache = {}


def make_in_maps(data, W1, W2, W3, W4, W5):
    bands = _build_bands(W1, W2, W3, W4, W5)
    dpad = np.zeros((B, D + 2 * HZ, D, D + 2), np.float32)
    dpad[:, HZ : HZ + D, :, 1 : D + 1] = data
    in_maps = []
    for c in range(8):
        bi, s = c // 4, c % 4
        in_maps.append(
            dict(xslab=np.ascontiguousarray(dpad[bi, s * ZS : s * ZS + ZIN]), **bands)
        )
    return in_maps


def kernel(data, W1, W2, W3, W4, W5):
    _ensure_import_path()
    data = np.asarray(data, np.float32)
    if "nc" not in _prog_cache:
        _prog_cache["nc"] = build_program()
    nc = _prog_cache["nc"]

    in_maps = make_in_maps(data, W1, W2, W3, W4, W5)
    res = bass_utils.run_bass_kernel_spmd(nc, in_maps, list(range(8))).results

    prob = np.zeros((B, 1, D, D, D), np.float32)
    regr = np.zeros((B, 1, D, D, D), np.float32)
    for c in range(8):
        bi, s = c // 4, c % 4
        prob[bi, 0, s * ZS : (s + 1) * ZS] = res[c]["prob_o"]
        regr[bi, 0, s * ZS : (s + 1) * ZS] = res[c]["regr_o"]
    return (prob, regr)


# revision 20
# speedup vs baseline: 4.4101x; 1.0075x over previous
"""Trainium2 Bass kernel for the 5-layer dilated sparse-conv encoder.

Network (per batch): 1ch -> [3x3x3 dil1] -> 2ch -> [3x3x3 dil2] -> 2ch
-> [3x3x3 dil4] -> 2ch -> [3x3x3 dil2] -> 2ch -> [1x1x1] -> sigmoid,
with relu+occupancy-mask after each hidden conv and mask after sigmoid.

Sharding: 8 cores = 2 batches x 4 z-slabs of 48 planes. Each core gets a
66-plane input slab (z halo 9) and computes its 48 output planes with no
cross-core communication.

Per-core algorithm: contraction over z on the TensorEngine. Every layer's
activation lives in SBUF as [2ch*64 partitions, y_window, 192] where the
64-row z-window is [z0-8, z1+8). A conv layer is 9 PSUM-accumulated
matmuls (one per (dy,dx) tap, shifted free-dim access patterns) against
host-built banded weight matrices that fold the 3 dz taps and both
channels into the contraction. Zero 'SAME' padding at volume borders
falls out of PSUM's has_written bits via per-tap restricted output
rectangles. relu+mask is one fused scalar_tensor_tensor DVE op.
"""

import os
import sys

import numpy as np


def _ensure_import_path():
    for p in ("/opt/trn_rl_repo", "/root/.axon_site/_ro/trn_rl_repo"):
        if os.path.isdir(p) and p not in sys.path:
            sys.path.insert(0, p)


_ensure_import_path()

import concourse.mybir as mybir  # noqa: E402
import concourse.tile as tile  # noqa: E402
from concourse import bacc, bass_utils  # noqa: E402

B, D = 2, 192
ZS = 48  # z planes per core
HZ = 9  # input z halo
ZIN = ZS + 2 * HZ  # 66 input planes per core
ZW = 64  # uniform stored z-window [z0-8, z1+8)
TAPS = [(0, 0)] + [
    (dy, dx) for dy in (-1, 0, 1) for dx in (-1, 0, 1) if (dy, dx) != (0, 0)
]
# (dilation, valid z-window in 64-coords) per layer
LAYERS = [(1, 0, 64), (2, 2, 62), (4, 6, 58), (2, 8, 56)]
V5 = (8, 56)

ACT_DT = "float32r"  # activation/storage dtype for conv tiles (full-rate PE)
YBLK = 16  # output-y rows per wavefront block


def _clip(a, b):
    return max(a, 0), min(b, D)


def _build_bands(W1, W2, W3, W4, W5):
    """Banded lhsT matrices, one per (layer, tap). Returns dict of arrays."""
    Ws = [np.asarray(w, np.float32) for w in (W1, W2, W3, W4)]
    out = {}
    # L1: [66, 9, 128]  (Cin=1)
    b1 = np.zeros((ZIN, 9, 128), np.float32)
    for t, (dy, dx) in enumerate(TAPS):
        for co in range(2):
            for dz in (-1, 0, 1):
                for zr in range(64):
                    b1[zr + 1 + dz, t, co * 64 + zr] = Ws[0][
                        co, 0, dz + 1, dy + 1, dx + 1
                    ]
    out["b1"] = b1
    # L2..L4: [128, 9, 128]
    for li, (d, a, b) in enumerate(LAYERS[1:], start=2):
        w = Ws[li - 1]
        bb = np.zeros((128, 9, 128), np.float32)
        for t, (dy, dx) in enumerate(TAPS):
            for co in range(2):
                for ci in range(2):
                    for dz in (-1, 0, 1):
                        for zr in range(a, b):
                            bb[ci * 64 + zr + d * dz, t, co * 64 + zr] = w[
                                co, ci, dz + 1, dy + 1, dx + 1
                            ]
        out[f"b{li}"] = bb
    # L5: [128, 128]
    w5 = np.asarray(W5, np.float32)
    b5 = np.zeros((128, 128), np.float32)
    for co in range(2):
        for ci in range(2):
            for zr in range(V5[0], V5[1]):
                b5[ci * 64 + zr, co * 64 + zr] = w5[co, ci, 0, 0, 0]
    out["b5"] = b5
    return out


def _conv_layer(nc, ps, src, src_y0, K, bt, d, wy, mk, mk_y0, dst, dst_y0,
                relu_mask=True, xpad=0):
    """One conv layer for one y-block: 9-tap banded matmuls + fused epilogue.

    src: source tile AP [K partitions, ylen, 192 + 2*xpad] whose row 0 is
    global y = src_y0 (x col j holds global x = j - xpad; with xpad the
    border columns are zeros and x is never range-restricted, which keeps
    fp32r ISA alignment happy for odd dilation shifts). dst computes global
    rows wy=(w0,w1), dst tile row 0 = global dst_y0. mk row 0 = mk_y0.
    """
    w0, w1 = wy
    taps = TAPS if bt.shape[1] == 9 else [(0, 0)]
    for ys in range(w0, w1, 8):
        ye = min(ys + 8, w1)
        for xs in range(0, D, 64):
            xe = xs + 64
            acc = ps.tile([128, 8, 64], mybir.dt.float32, tag="psum")
            live = []
            for t, (dy, dx) in enumerate(taps):
                oy0, oy1 = max(ys, -dy * d), min(ye, D - dy * d)
                if xpad:
                    ox0, ox1 = xs, xe
                else:
                    ox0, ox1 = max(xs, -dx * d), min(xe, D - dx * d)
                if oy0 < oy1 and ox0 < ox1:
                    live.append((t, dy, dx, oy0, oy1, ox0, ox1))
            assert live[0][0] == 0  # center tap first, covers full rect
            for i, (t, dy, dx, oy0, oy1, ox0, ox1) in enumerate(live):
                sy0, sy1 = oy0 + dy * d - src_y0, oy1 + dy * d - src_y0
                assert 0 <= sy0 < sy1 <= src.shape[1], (sy0, sy1, src.shape)
                sx0 = ox0 + dx * d + xpad
                assert 0 <= sx0 and sx0 + (ox1 - ox0) <= src.shape[2]
                nc.tensor.matmul(
                    acc[:, oy0 - ys : oy1 - ys, ox0 - xs : ox1 - xs],
                    bt[0:K, t, :] if bt.shape[1] == 9 else bt[0:K, :],
                    src[0:K, sy0:sy1, sx0 : sx0 + (ox1 - ox0)],
                    start=(i == 0),
                    stop=(i == len(live) - 1),
                )
            cy = ye - ys
            if relu_mask:
                nc.vector.scalar_tensor_tensor(
                    dst[:, ys - dst_y0 : ye - dst_y0, xs:xe],
                    acc[:, 0:cy, :],
                    0.0,
                    mk[:, ys - mk_y0 : ye - mk_y0, xs:xe],
                    op0=mybir.AluOpType.max,
                    op1=mybir.AluOpType.mult,
                )
            else:  # L5: sigmoid then mask
                nc.scalar.activation(
                    dst[:, ys - dst_y0 : ye - dst_y0, xs:xe],
                    acc[:, 0:cy, :],
                    mybir.ActivationFunctionType.Sigmoid,
                )
                nc.vector.tensor_tensor(
                    dst[:, ys - dst_y0 : ye - dst_y0, xs:xe],
                    dst[:, ys - dst_y0 : ye - dst_y0, xs:xe],
                    mk[:, ys - mk_y0 : ye - mk_y0, xs:xe],
                    op=mybir.AluOpType.mult,
                )


def build_program():
    dt = getattr(mybir.dt, ACT_DT)
    f32 = mybir.dt.float32
    nc = bacc.Bacc("TRN2", target_bir_lowering=False, debug=False)

    xslab = nc.dram_tensor("xslab", [ZIN, D, D + 2], dt, kind="ExternalInput")
    b1d = nc.dram_tensor("b1", [ZIN, 9, 128], dt, kind="ExternalInput")
    b2d = nc.dram_tensor("b2", [128, 9, 128], dt, kind="ExternalInput")
    b3d = nc.dram_tensor("b3", [128, 9, 128], dt, kind="ExternalInput")
    b4d = nc.dram_tensor("b4", [128, 9, 128], dt, kind="ExternalInput")
    b5d = nc.dram_tensor("b5", [128, 128], dt, kind="ExternalInput")
    prob_o = nc.dram_tensor("prob_o", [ZS, D, D], f32, kind="ExternalOutput")
    regr_o = nc.dram_tensor("regr_o", [ZS, D, D], f32, kind="ExternalOutput")

    # Skewed y-wavefront: layer l's computed frontier leads the output by
    # h_l rows. Block b computes rows [F(h,b-1), F(h,b)) of each layer
    # (YB rows interior, more in the prologue block, fewer in the last).
    # Each layer tile carries a 2*dil_consumer-row tail between blocks.
    HLEAD = {"xt": HZ, "t1": 8, "t2": 6, "t3": 2, "t4": 0}
    DCONS = {"xt": 1, "t1": 2, "t2": 4, "t3": 2, "t4": 0}

    def F(h, b):
        return 0 if b < 0 else min(YBLK * (b + 1) + h, D)

    def origin(h, dc, b):
        return 0 if b == 0 else F(h, b - 1) - 2 * dc

    NB = D // YBLK

    with tile.TileContext(nc) as tc:
        with (
            tc.tile_pool(name="wpool", bufs=1) as wp,
            tc.tile_pool(name="act", bufs=1) as ap,
            tc.tile_pool(name="mkp", bufs=2) as mkp,
            tc.tile_pool(name="otp", bufs=2) as otp,
            tc.tile_pool(name="ps", bufs=8, space="PSUM") as ps,
        ):
            b1t = wp.tile([ZIN, 9, 128], dt)
            b2t = wp.tile([128, 9, 128], dt)
            b3t = wp.tile([128, 9, 128], dt)
            b4t = wp.tile([128, 9, 128], dt)
            b5t = wp.tile([128, 128], dt)
            # b1 on the HWDGE path (needed first, ahead of block 0's input);
            # the rest via SWDGE so they don't queue ahead of it
            nc.sync.dma_start(b1t[:], b1d[:])
            for t, dram in ((b2t, b2d), (b3t, b3d), (b4t, b4d), (b5t, b5d)):
                nc.gpsimd.dma_start(t[:], dram[:])

            def ext(name):
                return max(YBLK + HLEAD[name], YBLK + 2 * DCONS[name])

            xt = ap.tile([ZIN, ext("xt"), D + 2], dt, tag="xt")
            t1 = ap.tile([128, ext("t1"), D], dt, tag="t1")
            t2 = ap.tile([128, ext("t2"), D], dt, tag="t2")
            t3 = ap.tile([128, ext("t3"), D], dt, tag="t3")
            t4 = ap.tile([128, ext("t4"), D], dt, tag="t4")

            for b in range(NB):
                # tail copies: last 2*dc computed rows -> tile rows [0, 2dc)
                if b > 0:
                    for tl, name, xw in ((xt, "xt", D + 2), (t1, "t1", D),
                                         (t2, "t2", D), (t3, "t3", D)):
                        h, dc = HLEAD[name], DCONS[name]
                        s0 = F(h, b - 1) - 2 * dc - origin(h, dc, b - 1)
                        nc.vector.tensor_copy(
                            tl[:, 0 : 2 * dc, 0:xw], tl[:, s0 : s0 + 2 * dc, 0:xw]
                        )

                # new input rows
                i0, i1 = F(HZ, b - 1), F(HZ, b)
                r0 = i0 - origin(HZ, 1, b)
                nc.sync.dma_start(xt[:, r0 : r0 + i1 - i0, :], xslab[:, i0:i1, :])

                # mask for this block's union of layer windows
                m0, m1 = YBLK * b, min(YBLK * b + YBLK + 8, D)
                mk = mkp.tile([128, YBLK + 8, D], f32, tag="mk")
                nc.sync.dma_start(
                    mk[0:64, 0 : m1 - m0, :],
                    xslab[1:65, m0:m1, 1 : D + 1].bitcast(f32),
                )
                nc.sync.dma_start(
                    mk[64:128, 0 : m1 - m0, :],
                    xslab[1:65, m0:m1, 1 : D + 1].bitcast(f32),
                )
                nc.vector.tensor_scalar(
                    mk[:, 0 : m1 - m0, :], mk[:, 0 : m1 - m0, :], 0.0, None,
                    op0=mybir.AluOpType.not_equal,
                )

                chain = (
                    ("xt", xt, ZIN, b1t, 1, "t1", t1, 1),
                    ("t1", t1, 128, b2t, 2, "t2", t2, 0),
                    ("t2", t2, 128, b3t, 4, "t3", t3, 0),
                    ("t3", t3, 128, b4t, 2, "t4", t4, 0),
                )
                for sname, stile, K, bt, dil, dname, dtile, xp in chain:
                    wy = (F(HLEAD[dname], b - 1), F(HLEAD[dname], b))
                    _conv_layer(
                        nc, ps, stile[:],
                        origin(HLEAD[sname], DCONS[sname], b), K, bt[:], dil,
                        wy, mk[:], m0, dtile[:],
                        origin(HLEAD[dname], DCONS[dname], b), xpad=xp,
                    )

                # L5: 1x1 conv + sigmoid + mask + store, in 8-row groups
                w5 = (F(0, b - 1), F(0, b))
                o4 = origin(0, 0, b)
                for ys in range(w5[0], w5[1], 8):
                    ye = min(ys + 8, w5[1])
                    ot = otp.tile([128, 8, D], f32, tag="ot")
                    for ps0 in range(ys, ye, 2):
                        ps1 = min(ps0 + 2, ye)
                        acc = ps.tile([128, 2, D], mybir.dt.float32, tag="psum")
                        nc.tensor.matmul(
                            acc[:, 0 : ps1 - ps0, :],
                            b5t[:, :],
                            t4[:, ps0 - o4 : ps1 - o4, :],
                            start=True,
                            stop=True,
                        )
                        nc.scalar.activation(
                            ot[:, ps0 - ys : ps1 - ys, :],
                            acc[:, 0 : ps1 - ps0, :],
                            mybir.ActivationFunctionType.Sigmoid,
                        )
                    nc.vector.tensor_tensor(
                        ot[:, 0 : ye - ys, :],
                        ot[:, 0 : ye - ys, :],
                        mk[:, ys - m0 : ye - m0, :],
                        op=mybir.AluOpType.mult,
                    )
                    nc.sync.dma_start(
                        prob_o[:, ys:ye, :], ot[8:56, 0 : ye - ys, :]
                    )
                    nc.sync.dma_start(
                        regr_o[:, ys:ye, :], ot[72:120, 0 : ye - ys, :]
                    )

    nc.compile()
    return nc


_prog_cache = {}


def make_in_maps(data, W1, W2, W3, W4, W5):
    bands = _build_bands(W1, W2, W3, W4, W5)
    dpad = np.zeros((B, D + 2 * HZ, D, D + 2), np.float32)
    dpad[:, HZ : HZ + D, :, 1 : D + 1] = data
    in_maps = []
    for c in range(8):
        bi, s = c // 4, c % 4
        in_maps.append(
            dict(xslab=np.ascontiguousarray(dpad[bi, s * ZS : s * ZS + ZIN]), **bands)
        )
    return in_maps


def kernel(data, W1, W2, W3, W4, W5):
    _ensure_import_path()
    data = np.asarray(data, np.float32)
    if "nc" not in _prog_cache:
        _prog_cache["nc"] = build_program()
    nc = _prog_cache["nc"]

    in_maps = make_in_maps(data, W1, W2, W3, W4, W5)
    res = bass_utils.run_bass_kernel_spmd(nc, in_maps, list(range(8))).results

    prob = np.zeros((B, 1, D, D, D), np.float32)
    regr = np.zeros((B, 1, D, D, D), np.float32)
    for c in range(8):
        bi, s = c // 4, c % 4
        prob[bi, 0, s * ZS : (s + 1) * ZS] = res[c]["prob_o"]
        regr[bi, 0, s * ZS : (s + 1) * ZS] = res[c]["regr_o"]
    return (prob, regr)


# revision 21
# speedup vs baseline: 4.4165x; 1.0014x over previous
"""Trainium2 Bass kernel for the 5-layer dilated sparse-conv encoder.

Network (per batch): 1ch -> [3x3x3 dil1] -> 2ch -> [3x3x3 dil2] -> 2ch
-> [3x3x3 dil4] -> 2ch -> [3x3x3 dil2] -> 2ch -> [1x1x1] -> sigmoid,
with relu+occupancy-mask after each hidden conv and mask after sigmoid.

Sharding: 8 cores = 2 batches x 4 z-slabs of 48 planes. Each core gets a
66-plane input slab (z halo 9) and computes its 48 output planes with no
cross-core communication.

Per-core algorithm: contraction over z on the TensorEngine. Every layer's
activation lives in SBUF as [2ch*64 partitions, y_window, 192] where the
64-row z-window is [z0-8, z1+8). A conv layer is 9 PSUM-accumulated
matmuls (one per (dy,dx) tap, shifted free-dim access patterns) against
host-built banded weight matrices that fold the 3 dz taps and both
channels into the contraction. Zero 'SAME' padding at volume borders
falls out of PSUM's has_written bits via per-tap restricted output
rectangles. relu+mask is one fused scalar_tensor_tensor DVE op.
"""

import os
import sys

import numpy as np


def _ensure_import_path():
    for p in ("/opt/trn_rl_repo", "/root/.axon_site/_ro/trn_rl_repo"):
        if os.path.isdir(p) and p not in sys.path:
            sys.path.insert(0, p)


_ensure_import_path()

import concourse.mybir as mybir  # noqa: E402
import concourse.tile as tile  # noqa: E402
from concourse import bacc, bass_utils  # noqa: E402

B, D = 2, 192
ZS = 48  # z planes per core
HZ = 9  # input z halo
ZIN = ZS + 2 * HZ  # 66 input planes per core
ZW = 64  # uniform stored z-window [z0-8, z1+8)
TAPS = [(0, 0)] + [
    (dy, dx) for dy in (-1, 0, 1) for dx in (-1, 0, 1) if (dy, dx) != (0, 0)
]
# (dilation, valid z-window in 64-coords) per layer
LAYERS = [(1, 0, 64), (2, 2, 62), (4, 6, 58), (2, 8, 56)]
V5 = (8, 56)

ACT_DT = "float32r"  # activation/storage dtype for conv tiles (full-rate PE)
YBLK = 16  # output-y rows per wavefront block


def _clip(a, b):
    return max(a, 0), min(b, D)


def _build_bands(W1, W2, W3, W4, W5):
    """Banded lhsT matrices, one per (layer, tap). Returns dict of arrays."""
    Ws = [np.asarray(w, np.float32) for w in (W1, W2, W3, W4)]
    zr = np.arange(64)
    tap_idx = [(dy + 1, dx + 1) for dy, dx in TAPS]
    out = {}
    # L1: [66, 9, 128]  (Cin=1): k = zr + 1 + dz, m = co*64 + zr
    b1 = np.zeros((ZIN, 9, 128), np.float32)
    for t, (iy, ix) in enumerate(tap_idx):
        for co in range(2):
            for dz in (-1, 0, 1):
                b1[zr + 1 + dz, t, co * 64 + zr] = Ws[0][co, 0, dz + 1, iy, ix]
    out["b1"] = b1
    # L2..L4: [128, 9, 128]: k = ci*64 + zr + d*dz, m = co*64 + zr
    for li, (d, a, b) in enumerate(LAYERS[1:], start=2):
        w = Ws[li - 1]
        zv = np.arange(a, b)
        bb = np.zeros((128, 9, 128), np.float32)
        for t, (iy, ix) in enumerate(tap_idx):
            for co in range(2):
                for ci in range(2):
                    for dz in (-1, 0, 1):
                        bb[ci * 64 + zv + d * dz, t, co * 64 + zv] = w[
                            co, ci, dz + 1, iy, ix
                        ]
        out[f"b{li}"] = bb
    # L5: [128, 128]
    w5 = np.asarray(W5, np.float32)
    b5 = np.zeros((128, 128), np.float32)
    zv = np.arange(V5[0], V5[1])
    for co in range(2):
        for ci in range(2):
            b5[ci * 64 + zv, co * 64 + zv] = w5[co, ci, 0, 0, 0]
    out["b5"] = b5
    return out


def _conv_layer(nc, ps, src, src_y0, K, bt, d, wy, mk, mk_y0, dst, dst_y0,
                relu_mask=True, xpad=0):
    """One conv layer for one y-block: 9-tap banded matmuls + fused epilogue.

    src: source tile AP [K partitions, ylen, 192 + 2*xpad] whose row 0 is
    global y = src_y0 (x col j holds global x = j - xpad; with xpad the
    border columns are zeros and x is never range-restricted, which keeps
    fp32r ISA alignment happy for odd dilation shifts). dst computes global
    rows wy=(w0,w1), dst tile row 0 = global dst_y0. mk row 0 = mk_y0.
    """
    w0, w1 = wy
    taps = TAPS if bt.shape[1] == 9 else [(0, 0)]
    for ys in range(w0, w1, 8):
        ye = min(ys + 8, w1)
        for xs in range(0, D, 64):
            xe = xs + 64
            acc = ps.tile([128, 8, 64], mybir.dt.float32, tag="psum")
            live = []
            for t, (dy, dx) in enumerate(taps):
                oy0, oy1 = max(ys, -dy * d), min(ye, D - dy * d)
                if xpad:
                    ox0, ox1 = xs, xe
                else:
                    ox0, ox1 = max(xs, -dx * d), min(xe, D - dx * d)
                if oy0 < oy1 and ox0 < ox1:
                    live.append((t, dy, dx, oy0, oy1, ox0, ox1))
            assert live[0][0] == 0  # center tap first, covers full rect
            for i, (t, dy, dx, oy0, oy1, ox0, ox1) in enumerate(live):
                sy0, sy1 = oy0 + dy * d - src_y0, oy1 + dy * d - src_y0
                assert 0 <= sy0 < sy1 <= src.shape[1], (sy0, sy1, src.shape)
                sx0 = ox0 + dx * d + xpad
                assert 0 <= sx0 and sx0 + (ox1 - ox0) <= src.shape[2]
                nc.tensor.matmul(
                    acc[:, oy0 - ys : oy1 - ys, ox0 - xs : ox1 - xs],
                    bt[0:K, t, :] if bt.shape[1] == 9 else bt[0:K, :],
                    src[0:K, sy0:sy1, sx0 : sx0 + (ox1 - ox0)],
                    start=(i == 0),
                    stop=(i == len(live) - 1),
                )
            cy = ye - ys
            if relu_mask:
                nc.vector.scalar_tensor_tensor(
                    dst[:, ys - dst_y0 : ye - dst_y0, xs:xe],
                    acc[:, 0:cy, :],
                    0.0,
                    mk[:, ys - mk_y0 : ye - mk_y0, xs:xe],
                    op0=mybir.AluOpType.max,
                    op1=mybir.AluOpType.mult,
                )
            else:  # L5: sigmoid then mask
                nc.scalar.activation(
                    dst[:, ys - dst_y0 : ye - dst_y0, xs:xe],
                    acc[:, 0:cy, :],
                    mybir.ActivationFunctionType.Sigmoid,
                )
                nc.vector.tensor_tensor(
                    dst[:, ys - dst_y0 : ye - dst_y0, xs:xe],
                    dst[:, ys - dst_y0 : ye - dst_y0, xs:xe],
                    mk[:, ys - mk_y0 : ye - mk_y0, xs:xe],
                    op=mybir.AluOpType.mult,
                )


def build_program():
    dt = getattr(mybir.dt, ACT_DT)
    f32 = mybir.dt.float32
    nc = bacc.Bacc("TRN2", target_bir_lowering=False, debug=False)

    xslab = nc.dram_tensor("xslab", [ZIN, D, D + 2], dt, kind="ExternalInput")
    b1d = nc.dram_tensor("b1", [ZIN, 9, 128], dt, kind="ExternalInput")
    b2d = nc.dram_tensor("b2", [128, 9, 128], dt, kind="ExternalInput")
    b3d = nc.dram_tensor("b3", [128, 9, 128], dt, kind="ExternalInput")
    b4d = nc.dram_tensor("b4", [128, 9, 128], dt, kind="ExternalInput")
    b5d = nc.dram_tensor("b5", [128, 128], dt, kind="ExternalInput")
    prob_o = nc.dram_tensor("prob_o", [ZS, D, D], f32, kind="ExternalOutput")
    regr_o = nc.dram_tensor("regr_o", [ZS, D, D], f32, kind="ExternalOutput")

    # Skewed y-wavefront: layer l's computed frontier leads the output by
    # h_l rows. Block b computes rows [F(h,b-1), F(h,b)) of each layer
    # (YB rows interior, more in the prologue block, fewer in the last).
    # Each layer tile carries a 2*dil_consumer-row tail between blocks.
    HLEAD = {"xt": HZ, "t1": 8, "t2": 6, "t3": 2, "t4": 0}
    DCONS = {"xt": 1, "t1": 2, "t2": 4, "t3": 2, "t4": 0}

    def F(h, b):
        return 0 if b < 0 else min(YBLK * (b + 1) + h, D)

    def origin(h, dc, b):
        return 0 if b == 0 else F(h, b - 1) - 2 * dc

    NB = D // YBLK

    with tile.TileContext(nc) as tc:
        with (
            tc.tile_pool(name="wpool", bufs=1) as wp,
            tc.tile_pool(name="act", bufs=1) as ap,
            tc.tile_pool(name="mkp", bufs=2) as mkp,
            tc.tile_pool(name="otp", bufs=2) as otp,
            tc.tile_pool(name="ps", bufs=8, space="PSUM") as ps,
        ):
            b1t = wp.tile([ZIN, 9, 128], dt)
            b2t = wp.tile([128, 9, 128], dt)
            b3t = wp.tile([128, 9, 128], dt)
            b4t = wp.tile([128, 9, 128], dt)
            b5t = wp.tile([128, 128], dt)
            # b1 on the HWDGE path (needed first, ahead of block 0's input);
            # the rest via SWDGE so they don't queue ahead of it
            nc.sync.dma_start(b1t[:], b1d[:])
            for t, dram in ((b2t, b2d), (b3t, b3d), (b4t, b4d), (b5t, b5d)):
                nc.gpsimd.dma_start(t[:], dram[:])

            def ext(name):
                return max(YBLK + HLEAD[name], YBLK + 2 * DCONS[name])

            xt = ap.tile([ZIN, ext("xt"), D + 2], dt, tag="xt")
            t1 = ap.tile([128, ext("t1"), D], dt, tag="t1")
            t2 = ap.tile([128, ext("t2"), D], dt, tag="t2")
            t3 = ap.tile([128, ext("t3"), D], dt, tag="t3")
            t4 = ap.tile([128, ext("t4"), D], dt, tag="t4")

            for b in range(NB):
                # tail copies: last 2*dc computed rows -> tile rows [0, 2dc)
                if b > 0:
                    for tl, name, xw in ((xt, "xt", D + 2), (t1, "t1", D),
                                         (t2, "t2", D), (t3, "t3", D)):
                        h, dc = HLEAD[name], DCONS[name]
                        s0 = F(h, b - 1) - 2 * dc - origin(h, dc, b - 1)
                        nc.vector.tensor_copy(
                            tl[:, 0 : 2 * dc, 0:xw], tl[:, s0 : s0 + 2 * dc, 0:xw]
                        )

                # new input rows
                i0, i1 = F(HZ, b - 1), F(HZ, b)
                r0 = i0 - origin(HZ, 1, b)
                nc.sync.dma_start(xt[:, r0 : r0 + i1 - i0, :], xslab[:, i0:i1, :])

                # mask for this block's union of layer windows
                m0, m1 = YBLK * b, min(YBLK * b + YBLK + 8, D)
                mk = mkp.tile([128, YBLK + 8, D], f32, tag="mk")
                nc.sync.dma_start(
                    mk[0:64, 0 : m1 - m0, :],
                    xslab[1:65, m0:m1, 1 : D + 1].bitcast(f32),
                )
                nc.sync.dma_start(
                    mk[64:128, 0 : m1 - m0, :],
                    xslab[1:65, m0:m1, 1 : D + 1].bitcast(f32),
                )
                nc.vector.tensor_scalar(
                    mk[:, 0 : m1 - m0, :], mk[:, 0 : m1 - m0, :], 0.0, None,
                    op0=mybir.AluOpType.not_equal,
                )

                chain = (
                    ("xt", xt, ZIN, b1t, 1, "t1", t1, 1),
                    ("t1", t1, 128, b2t, 2, "t2", t2, 0),
                    ("t2", t2, 128, b3t, 4, "t3", t3, 0),
                    ("t3", t3, 128, b4t, 2, "t4", t4, 0),
                )
                for sname, stile, K, bt, dil, dname, dtile, xp in chain:
                    wy = (F(HLEAD[dname], b - 1), F(HLEAD[dname], b))
                    _conv_layer(
                        nc, ps, stile[:],
                        origin(HLEAD[sname], DCONS[sname], b), K, bt[:], dil,
                        wy, mk[:], m0, dtile[:],
                        origin(HLEAD[dname], DCONS[dname], b), xpad=xp,
                    )

                # L5: 1x1 conv + sigmoid + mask + store, in 8-row groups
                w5 = (F(0, b - 1), F(0, b))
                o4 = origin(0, 0, b)
                for ys in range(w5[0], w5[1], 8):
                    ye = min(ys + 8, w5[1])
                    ot = otp.tile([128, 8, D], f32, tag="ot")
                    for ps0 in range(ys, ye, 2):
                        ps1 = min(ps0 + 2, ye)
                        acc = ps.tile([128, 2, D], mybir.dt.float32, tag="psum")
                        nc.tensor.matmul(
                            acc[:, 0 : ps1 - ps0, :],
                            b5t[:, :],
                            t4[:, ps0 - o4 : ps1 - o4, :],
                            start=True,
                            stop=True,
                        )
                        nc.scalar.activation(
                            ot[:, ps0 - ys : ps1 - ys, :],
                            acc[:, 0 : ps1 - ps0, :],
                            mybir.ActivationFunctionType.Sigmoid,
                        )
                    nc.vector.tensor_tensor(
                        ot[:, 0 : ye - ys, :],
                        ot[:, 0 : ye - ys, :],
                        mk[:, ys - m0 : ye - m0, :],
                        op=mybir.AluOpType.mult,
                    )
                    nc.sync.dma_start(
                        prob_o[:, ys:ye, :], ot[8:56, 0 : ye - ys, :]
                    )
                    nc.sync.dma_start(
                        regr_o[:, ys:ye, :], ot[72:120, 0 : ye - ys, :]
                    )

    nc.compile()
    return nc


_prog_cache = {}


def make_in_maps(data, W1, W2, W3, W4, W5):
    bands = _build_bands(W1, W2, W3, W4, W5)
    dpad = np.zeros((B, D + 2 * HZ, D, D + 2), np.float32)
    dpad[:, HZ : HZ + D, :, 1 : D + 1] = data
    in_maps = []
    for c in range(8):
        bi, s = c // 4, c % 4
        in_maps.append(
            dict(xslab=np.ascontiguousarray(dpad[bi, s * ZS : s * ZS + ZIN]), **bands)
        )
    return in_maps


def kernel(data, W1, W2, W3, W4, W5):
    _ensure_import_path()
    data = np.asarray(data, np.float32)
    if "nc" not in _prog_cache:
        _prog_cache["nc"] = build_program()
    nc = _prog_cache["nc"]

    in_maps = make_in_maps(data, W1, W2, W3, W4, W5)
    res = bass_utils.run_bass_kernel_spmd(nc, in_maps, list(range(8))).results

    prob = np.zeros((B, 1, D, D, D), np.float32)
    regr = np.zeros((B, 1, D, D, D), np.float32)
    for c in range(8):
        bi, s = c // 4, c % 4
        prob[bi, 0, s * ZS : (s + 1) * ZS] = res[c]["prob_o"]
        regr[bi, 0, s * ZS : (s + 1) * ZS] = res[c]["regr_o"]
    return (prob, regr)
